# revision 27
# baseline (speedup 1.0000x reference)
"""Trainium2 Bass kernel for CausalSelfAttention (PentaNet-quantized weights).

Reference computation (B=2, T=2048, C=1024, H=16 heads, D=64):
    qkv = x @ quant(w_attn).T ; split q,k,v ; causal softmax attention ;
    out = y @ quant(w_proj).T

Sharding: 8 cores = 2 (batch) x 4 (head groups of 4 heads).  Each core
computes its batch element's attention for its 4 heads plus the partial
output projection over its 256 input channels; the host sums the 4
partials per batch (the w_proj contraction is split across head groups).

Device layout (v2 - transposed attention-value product):
  - host supplies xT = x[b].T  [C, T]
  - qT,kT computed as [d, t] (weights stationary), v as [t, d]
  - scores transposed: ST[j, i] = k_j . q_i  (j = key pos)
  - P = exp(ST/8) with causal masking (block-skip + triangular mask)
  - AV transposed *again*: O[i, d] = sum_j P[j, i] V[j, d] with P as the
    matmul stationary ([128 j, 128 i] tiles) and V [128 j, 65 d] moving.
    The 65-wide streams put the short dim on the clock and the full 128
    partitions on output -> half the PE time of the [65, i]-oriented
    product.  V's extra ones-column makes O[:, 64] the softmax
    denominator, per-*partition*, so normalization is a reciprocal plus
    a tensor_scalar multiply (no partition broadcast needed).
  - y[i, d] tiles (two heads side by side, [128 i, 128 d2]) are
    DMA-XBAR-transposed into yt [128 d2, i] - exactly the lhsT layout the
    projection needs.  Projection partials stream out per t-tile in bf16;
    the host sums the 4 head-group partials per batch in fp32.
All matmuls run in bf16 (fp32 PSUM accumulation); measured end-to-end
rel err vs the fp32 reference is ~5e-3.
"""

import os
import sys

sys.path.insert(0, "/opt/trn_rl_repo")

import numpy as np
import ml_dtypes

import jax

try:
    jax.config.update("jax_compilation_cache_dir", "/root/.cache/jax_bass_neff")
except Exception:
    pass

import concourse.bass as bass
import concourse.tile as tile
from concourse import bacc, mybir
from concourse.bass_utils import run_bass_kernel_spmd

F32 = mybir.dt.float32
F32R = mybir.dt.float32r
BF16 = mybir.dt.bfloat16

B, T, C = 2, 2048, 1024
H, D = 16, 64
HL = 4                    # heads per core
OL = HL * D               # 256 local output channels
KT = C // 128             # 8 k-tiles over C
TT = T // 128             # 16 t-tiles
NCH = T // 512            # 4 i-chunks of 512
SCALE = 1.0 / 8.0         # 1/sqrt(D)


def build_body(ctx, tc, xT, wq, wk, wv, wp, tri, onesd, out):
    nc = tc.nc

    consts = ctx.enter_context(tc.tile_pool(name="consts", bufs=1))
    acts = ctx.enter_context(tc.tile_pool(name="acts", bufs=1))
    pp = ctx.enter_context(tc.tile_pool(name="pp", bufs=4))
    rcp = ctx.enter_context(tc.tile_pool(name="rcp", bufs=2))
    y2p = ctx.enter_context(tc.tile_pool(name="y2p", bufs=8))
    obp = ctx.enter_context(tc.tile_pool(name="obp", bufs=5))
    ps_mm = ctx.enter_context(tc.tile_pool(name="ps_mm", bufs=2, space="PSUM"))
    ps_pj = ctx.enter_context(tc.tile_pool(name="ps_pj", bufs=2, space="PSUM"))
    ps_ot = ctx.enter_context(tc.tile_pool(name="ps_ot", bufs=2, space="PSUM"))

    # ---- SBUF tiles ----
    wq_sb = consts.tile([128, KT * OL], BF16)
    wk_sb = consts.tile([128, KT * OL], BF16)
    wv_sb = consts.tile([128, KT * OL], BF16)
    xT_sb = consts.tile([128, KT * T], BF16)
    tri_sb = consts.tile([128, 128], BF16)
    wp_sb = consts.tile([128, 2 * C], BF16)

    q_sb = acts.tile([128, 2 * T], BF16)
    k_sb = acts.tile([128, 2 * T], BF16)
    v_sb = acts.tile([128, TT * HL * (D + 1)], BF16)
    yt_sb = acts.tile([128, 2 * T], BF16)

    # ---- input DMA ----
    # Host supplies all tensors pre-tiled to the exact SBUF layouts, so every
    # load is a contiguous full-rate DMA.  x is chunk-major: [128, NCH, KT, 512].
    def load_w(w_sb, w_d, k0, k1):
        nc.sync.dma_start(w_sb[:, k0 * OL:k1 * OL], w_d[:, k0 * OL:k1 * OL])

    def load_x(n, k0, k1):
        nc.sync.dma_start(
            xT_sb[:, (n * KT + k0) * 512:(n * KT + k1) * 512],
            xT[:, (n * KT + k0) * 512:(n * KT + k1) * 512])

    load_w(wq_sb, wq, 0, 2)
    load_x(0, 0, 2)
    load_w(wq_sb, wq, 2, KT)
    load_x(0, 2, KT)
    load_w(wv_sb, wv, 0, KT)
    load_w(wk_sb, wk, 0, KT)
    load_x(1, 0, KT)
    nc.sync.dma_start(tri_sb[:], tri[:, :])
    # ones column (index D) of every [t-tile, head] V block
    v_ones = v_sb[:].rearrange("p (g c) -> p g c", c=D + 1)[:, :, D]
    nc.sync.dma_start(v_ones, onesd[:, :])
    load_x(2, 0, KT)
    load_x(3, 0, KT)
    nc.sync.dma_start(wp_sb[:], wp[:, :])

    # ---- qkv / proj emission units ----
    def qk_unit(n, which, m):
        w_sb, dst = (wq_sb, q_sb) if which == 0 else (wk_sb, k_sb)
        ps = ps_pj.tile([128, 512], F32, tag="pj")
        for k in range(KT):
            nc.tensor.matmul(
                ps[:],
                w_sb[:, k * OL + m * 128: k * OL + (m + 1) * 128],
                xT_sb[:, (n * KT + k) * 512:(n * KT + k + 1) * 512],
                start=(k == 0), stop=(k == KT - 1),
            )
        nc.vector.tensor_copy(dst[:, m * T + n * 512: m * T + (n + 1) * 512], ps[:])

    def v_unit(t):
        ps = ps_pj.tile([128, OL], F32, tag="pj")
        n, tl = divmod(t, 4)
        for k in range(KT):
            nc.tensor.matmul(
                ps[:],
                xT_sb[:, (n * KT + k) * 512 + tl * 128:
                       (n * KT + k) * 512 + (tl + 1) * 128],
                wv_sb[:, k * OL:(k + 1) * OL],
                start=(k == 0), stop=(k == KT - 1),
            )
        dst = v_sb[:, t * HL * (D + 1): (t + 1) * HL * (D + 1)]
        dst = dst.rearrange("p (h c) -> p h c", h=HL)[:, :, 0:D]
        nc.vector.tensor_copy(dst, ps[:].rearrange("p (h c) -> p h c", h=HL))

    _ob_state = {}

    def proj_unit(t, n2):
        ps = ps_pj.tile([128, 512], F32, tag="pj")
        for kk in range(2):
            nc.tensor.matmul(
                ps[:],
                yt_sb[:, kk * T + t * 128: kk * T + (t + 1) * 128],
                wp_sb[:, kk * C + n2 * 512: kk * C + (n2 + 1) * 512],
                start=(kk == 0), stop=(kk == 1),
            )
        # pair two t-tiles per output store to halve DMA dispatch count,
        # except the last two tiles (tail latency: let t14's store overlap
        # t15's projection)
        if t >= TT - 2:
            if n2 == 0:
                ob = obp.tile([128, 1024], BF16, tag="ob", name=f"ob_s{t}")
                nc.scalar.copy(ob[:, 0:512], ps[:])
                _ob_state[t] = ob
            else:
                ob = _ob_state.pop(t)
                nc.vector.tensor_copy(ob[:, 512:1024], ps[:])
                nc.sync.dma_start(out[t * 128:(t + 1) * 128, :], ob[:])
            return
        tp = t // 2
        ob = _ob_state.get(tp)
        if ob is None:
            ob = obp.tile([128, 2, 1024], BF16, tag="ob", name=f"ob_{tp}")
            _ob_state[tp] = ob
        cp = nc.scalar.copy if t >= 12 and n2 == 0 else nc.vector.tensor_copy
        if n2 == 0:
            cp(ob[:, t % 2, 0:512], ps[:])
        else:
            cp(ob[:, t % 2, 512:1024], ps[:])
            if t % 2 == 1:
                nc.sync.dma_start(
                    out[(t - 1) * 128:(t + 1) * 128, :]
                    .rearrange("(g p) c -> p g c", g=2),
                    ob[:])
                del _ob_state[tp]

    def qk_units(n):
        return [(lambda n=n, w=w, m=m: qk_unit(n, w, m))
                for w in range(2) for m in range(2)]

    def v_units(n):
        return [(lambda t=t: v_unit(t)) for t in range(4 * n, 4 * n + 4)]

    # ---- attention ----
    # state per (h, ic): ps_o [128 i, 4 itl, 65] accumulators + rc [128, 4]
    # state per ic: y4 [128 i, 4 itl x 256 d2] normalized outputs (all heads)
    _y4 = {}

    def attn_head_blocks(ic, h, post_diag0=lambda: None):
        pair, hl = h // 2, h % 2
        pb = 64 * hl
        mo = pair * T
        state = {}

        def ot_mms(kt, pieces, p_t):
            # pieces: list of (itl, col_off); emits stationary-P matmuls.
            # PSUM start_tensor_calc zeroes the whole 2KB bank (lazily), so
            # only the FIRST matmul into the bank may set start=True; the
            # other i-tile groups' first writes land on pending-zero bytes
            # and are zeroed by the hardware as part of that one start.
            for idx, (itl, co) in enumerate(pieces):
                nc.tensor.matmul(
                    state["ps_o"][:, itl, 0:D + 1],
                    p_t[:, co:co + 128],
                    v_sb[:, (kt * HL + h) * (D + 1):(kt * HL + h + 1) * (D + 1)],
                    start=(kt == 0 and idx == 0), stop=(kt == 4 * ic + itl),
                    skip_group_check=True,
                )

        def norm2(a):
            # normalize i-tiles a, a+1 in one reciprocal + one broadcast mul
            if a == 0:
                state["rc"] = rcp.tile([128, 4], F32, tag="rc", name=f"rc_{ic}_{h}")
            rc = state["rc"]
            ps_o = state["ps_o"]
            y4 = _y4.get(ic)
            if y4 is None:
                y4 = y2p.tile([128, 1024], BF16, tag="y4", name=f"y4_{ic}")
                _y4[ic] = y4
            nc.vector.reciprocal(rc[:, a:a + 2], ps_o[:, a:a + 2, D])
            dst = (y4[:].rearrange("p (i c) -> p i c", i=4)
                   [:, a:a + 2, h * 64:(h + 1) * 64])
            nc.vector.tensor_mul(dst, ps_o[:, a:a + 2, 0:D],
                                 rc[:, a:a + 2].to_broadcast([128, 2, D]))

        def transpose(itl):
            # one XBAR transpose covers both head pairs: [128 i, 256 d] ->
            # [256 d, 128 i] landing as kk=0/1 blocks of yt
            t = 4 * ic + itl
            y4 = _y4[ic]
            dst = yt_sb[:].rearrange("p (g t) -> p g t", g=2)[:, :, t * 128:(t + 1) * 128]
            nc.sync.dma_start_transpose(dst, y4[:, itl * 256:(itl + 1) * 256])
            if itl == 3:
                del _y4[ic]


        def full_pair(tja):
            if tja == 0:
                state["ps_o"] = ps_ot.tile([128, 4, 128], F32, tag="ot",
                                           name=f"ps_o_{ic}_{h}")
            qh = q_sb[pb:pb + 64, mo:mo + T]
            kh = k_sb[pb:pb + 64, mo:mo + T]
            ps_s = ps_mm.tile([128, 1024], F32, tag="mm", name=f"ps_s_{ic}_{h}")
            for j in range(2):
                nc.tensor.matmul(
                    ps_s[:, j * 512:(j + 1) * 512],
                    kh[:, (tja + j) * 128:(tja + j + 1) * 128],
                    qh[:, ic * 512:(ic + 1) * 512],
                    start=True, stop=True,
                    skip_group_check=True,
                )
            p_t = pp.tile([128, 1024], BF16, tag="p", name=f"p_t_{ic}_{h}")
            nc.scalar.activation(p_t[:], ps_s[:], mybir.ActivationFunctionType.Exp,
                                 scale=SCALE)
            for j in range(2):
                ot_mms(tja + j, [(itl, j * 512 + itl * 128) for itl in range(4)],
                       p_t)

        def diag0a():
            if ic == 0:
                state["ps_o"] = ps_ot.tile([128, 4, 128], F32, tag="ot",
                                           name=f"ps_o_{ic}_{h}")
            qh = q_sb[pb:pb + 64, mo:mo + T]
            kh = k_sb[pb:pb + 64, mo:mo + T]
            kt = 4 * ic
            ps_s = ps_mm.tile([128, 1024], F32, tag="mm", name=f"ps_d0_{ic}_{h}")
            nc.tensor.matmul(
                ps_s[:, 0:512], kh[:, kt * 128:(kt + 1) * 128],
                qh[:, ic * 512:(ic + 1) * 512],
                start=True, stop=True, skip_group_check=True)
            nc.tensor.matmul(
                ps_s[:, 512:896], kh[:, (kt + 1) * 128:(kt + 2) * 128],
                qh[:, ic * 512 + 128:(ic + 1) * 512],
                start=True, stop=True, skip_group_check=True)
            p_t = pp.tile([128, 1024], BF16, tag="p", name=f"p_d0_{ic}_{h}")
            nc.scalar.activation(p_t[:, 0:896], ps_s[:, 0:896],
                                 mybir.ActivationFunctionType.Exp, scale=SCALE)
            nc.gpsimd.tensor_mul(p_t[:, 0:128], p_t[:, 0:128], tri_sb[:])
            nc.gpsimd.tensor_mul(p_t[:, 512:640], p_t[:, 512:640], tri_sb[:])
            state["p_d0"] = p_t

        def diag0b():
            kt = 4 * ic
            p_t = state.pop("p_d0")
            ot_mms(kt, [(itl, itl * 128) for itl in range(4)], p_t)
            ot_mms(kt + 1, [(itl, 512 + (itl - 1) * 128) for itl in (1, 2, 3)], p_t)
            norm2(0)
            if h == 3:
                transpose(0)
                transpose(1)
                post_diag0()

        def diag2a():
            qh = q_sb[pb:pb + 64, mo:mo + T]
            kh = k_sb[pb:pb + 64, mo:mo + T]
            kt = 4 * ic + 2
            ps_s = ps_mm.tile([128, 1024], F32, tag="mm", name=f"ps_d2_{ic}_{h}")
            nc.tensor.matmul(
                ps_s[:, 0:256], kh[:, kt * 128:(kt + 1) * 128],
                qh[:, ic * 512 + 256:(ic + 1) * 512],
                start=True, stop=True, skip_group_check=True)
            nc.tensor.matmul(
                ps_s[:, 256:384], kh[:, (kt + 1) * 128:(kt + 2) * 128],
                qh[:, ic * 512 + 384:(ic + 1) * 512],
                start=True, stop=True, skip_group_check=True)
            p_t = pp.tile([128, 1024], BF16, tag="p", name=f"p_d2_{ic}_{h}")
            nc.scalar.activation(p_t[:, 0:384], ps_s[:, 0:384],
                                 mybir.ActivationFunctionType.Exp, scale=SCALE)
            nc.gpsimd.tensor_mul(p_t[:, 0:128], p_t[:, 0:128], tri_sb[:])
            nc.gpsimd.tensor_mul(p_t[:, 256:384], p_t[:, 256:384], tri_sb[:])
            state["p_d2"] = p_t

        def diag2b():
            kt = 4 * ic + 2
            p_t = state.pop("p_d2")
            ot_mms(kt, [(2, 0), (3, 128)], p_t)
            ot_mms(kt + 1, [(3, 256)], p_t)
            norm2(2)
            if h == 3:
                transpose(2)
                transpose(3)

        blocks = []
        for tja in range(0, 4 * ic, 2):
            blocks.append(lambda tja=tja: full_pair(tja))
        blocks += [diag0a, diag0b, diag2a, diag2b]
        return blocks

    def emit_interleaved(blocks, fillers):
        nf, nb, fi = len(fillers), len(blocks), 0
        for i, blk in enumerate(blocks):
            blk()
            want = (i + 1) * nf // nb
            while fi < want:
                fillers[fi]()
                fi += 1
        while fi < nf:
            fillers[fi]()
            fi += 1

    def proj_units(ic, t0=0, t1=4):
        return [(lambda t=t, n2=n2: proj_unit(t, n2))
                for t in range(4 * ic + t0, 4 * ic + t1) for n2 in range(2)]

    # schedule: qkv(0) first; chunk ic interleaves qkv(ic+1) and deferred
    # projection work as PE filler.  The late chunks are locally ACT-bound
    # (exp is the binding resource there), so they get extra PE work: the
    # chunk-3 v-units, the back half of proj(1), and proj(2).  The chunk-3
    # projections for t12/t13 slide into head 3's diag0 slot, leaving only
    # t14/t15 (plus stores) as the serial tail.
    for u in qk_units(0) + v_units(0):
        u()
    for ic in range(NCH):
        blocks = []
        if ic == 3:
            blocks += v_units(3)
        post = lambda: None
        if ic == 3:
            def post():
                for u in proj_units(3, 0, 2):
                    u()
        for h in range(4):
            blocks += attn_head_blocks(ic, h, post_diag0=(post if h == 3 else (lambda: None)))
        fill = []
        if ic < 2:
            fill += qk_units(ic + 1) + v_units(ic + 1)
        elif ic == 2:
            fill += qk_units(3) + proj_units(1, 0, 2)
        if ic == 1:
            fill += proj_units(0)
        elif ic == 3:
            fill += proj_units(1, 2, 4) + proj_units(2)
        emit_interleaved(blocks, fill)
    for u in proj_units(3, 2, 4):
        u()


def build_program(reps=1):
    from contextlib import ExitStack

    nc = bacc.Bacc("TRN2", target_bir_lowering=False, debug=False)
    # all inputs pre-tiled by the host to SBUF layouts (partition dim first)
    xT = nc.dram_tensor("xT", [128, NCH * KT * 512], BF16, kind="ExternalInput").ap()
    wq = nc.dram_tensor("wq", [128, KT * OL], BF16, kind="ExternalInput").ap()
    wk = nc.dram_tensor("wk", [128, KT * OL], BF16, kind="ExternalInput").ap()
    wv = nc.dram_tensor("wv", [128, KT * OL], BF16, kind="ExternalInput").ap()
    wp = nc.dram_tensor("wp", [128, 2 * C], BF16, kind="ExternalInput").ap()
    tri = nc.dram_tensor("tri", [128, 128], BF16, kind="ExternalInput").ap()
    onesd = nc.dram_tensor("onesd", [128, TT * HL], BF16, kind="ExternalInput").ap()
    out = nc.dram_tensor("out", [T, C], BF16, kind="ExternalOutput").ap()

    with tile.TileContext(nc) as tc:
        for _ in range(reps):
            with ExitStack() as ctx:
                build_body(ctx, tc, xT, wq, wk, wv, wp, tri, onesd, out)
    nc.compile()
    return nc


def quant_weight_np(w):
    scale = max(np.mean(np.abs(w), dtype=np.float32), np.float32(1e-8))
    return (np.clip(np.round(w / scale), -2.0, 2.0) * scale).astype(np.float32)


def _tile_w(w):
    # [C, OL] -> [128, KT*OL]: SBUF layout, k-tile major along free dim
    return np.ascontiguousarray(
        w.reshape(KT, 128, OL).transpose(1, 0, 2).reshape(128, KT * OL))


def _tile_x(xTb):
    # [C, T] -> [128, NCH*KT*512]: chunk-major, then k-tile, then 512 cols
    v = xTb.reshape(KT, 128, NCH, 512)        # [k, p, n, col]
    return np.ascontiguousarray(
        v.transpose(1, 2, 0, 3).reshape(128, NCH * KT * 512))


def make_in_maps(x, w_attn, w_proj):
    wq_f = quant_weight_np(w_attn)
    wp_f = quant_weight_np(w_proj)
    tri = np.triu(np.ones((128, 128), dtype=np.float32))
    in_maps = []
    for core in range(8):
        b, g = divmod(core, 4)
        sl = slice(g * OL, (g + 1) * OL)
        wp_l = wp_f[:, sl].T                 # [OL, C]
        wp_t = np.ascontiguousarray(
            wp_l.reshape(2, 128, C).transpose(1, 0, 2).reshape(128, 2 * C))
        in_maps.append({
            "xT": _tile_x(x[b].T).astype(ml_dtypes.bfloat16),
            "wq": _tile_w(wq_f[0 * C:1 * C][sl].T).astype(ml_dtypes.bfloat16),
            "wk": _tile_w(wq_f[1 * C:2 * C][sl].T).astype(ml_dtypes.bfloat16),
            "wv": _tile_w(wq_f[2 * C:3 * C][sl].T).astype(ml_dtypes.bfloat16),
            "wp": wp_t.astype(ml_dtypes.bfloat16),
            "tri": tri.astype(ml_dtypes.bfloat16),
            "onesd": np.ones((128, TT * HL), dtype=ml_dtypes.bfloat16),
        })
    return in_maps


_CACHED_NC = None


def kernel(x, w_attn, w_proj):
    global _CACHED_NC
    if _CACHED_NC is None:
        _CACHED_NC = build_program()
    in_maps = make_in_maps(np.asarray(x, dtype=np.float32),
                           np.asarray(w_attn, dtype=np.float32),
                           np.asarray(w_proj, dtype=np.float32))
    res = run_bass_kernel_spmd(_CACHED_NC, in_maps, list(range(8)))
    out = np.zeros((B, T, C), dtype=np.float32)
    for core in range(8):
        b = core // 4
        out[b] += res.results[core]["out"].astype(np.float32)
    return out


# revision 38
# speedup vs baseline: 2.0079x; 2.0079x over previous
"""Trainium2 Bass kernel for CausalSelfAttention (PentaNet-quantized weights).

Reference computation (B=2, T=2048, C=1024, H=16 heads, D=64):
    qkv = x @ quant(w_attn).T ; split q,k,v ; causal softmax attention ;
    out = y @ quant(w_proj).T

Sharding: 8 cores = 2 (batch) x 4 (head groups of 4 heads).  Each core
computes its batch element's attention for its 4 heads plus the partial
output projection over its 256 input channels; the host sums the 4
partials per batch (the w_proj contraction is split across head groups).

Device layout (v2 - transposed attention-value product):
  - host supplies xT = x[b].T  [C, T]
  - qT,kT computed as [d, t] (weights stationary), v as [t, d]
  - scores transposed: ST[j, i] = k_j . q_i  (j = key pos)
  - P = exp(ST/8) with causal masking (block-skip + triangular mask)
  - AV transposed *again*: O[i, d] = sum_j P[j, i] V[j, d] with P as the
    matmul stationary ([128 j, 128 i] tiles) and V [128 j, 65 d] moving.
    The 65-wide streams put the short dim on the clock and the full 128
    partitions on output -> half the PE time of the [65, i]-oriented
    product.  V's extra ones-column makes O[:, 64] the softmax
    denominator, per-*partition*, so normalization is a reciprocal plus
    a tensor_scalar multiply (no partition broadcast needed).
  - y[i, d] tiles (two heads side by side, [128 i, 128 d2]) are
    DMA-XBAR-transposed into yt [128 d2, i] - exactly the lhsT layout the
    projection needs.  Projection partials stream out per t-tile in bf16;
    the host sums the 4 head-group partials per batch in fp32.
All matmuls run in bf16 (fp32 PSUM accumulation); measured end-to-end
rel err vs the fp32 reference is ~5e-3.
"""

import os
import sys

sys.path.insert(0, "/opt/trn_rl_repo")

import numpy as np
import ml_dtypes

import jax

try:
    jax.config.update("jax_compilation_cache_dir", "/root/.cache/jax_bass_neff")
except Exception:
    pass

import concourse.bass as bass
import concourse.tile as tile
from concourse import bacc, mybir
from concourse.bass_utils import run_bass_kernel_spmd

F32 = mybir.dt.float32
F32R = mybir.dt.float32r
BF16 = mybir.dt.bfloat16

B, T, C = 2, 2048, 1024
H, D = 16, 64
HL = 4                    # heads per core
OL = HL * D               # 256 local output channels
KT = C // 128             # 8 k-tiles over C
TT = T // 128             # 16 t-tiles
NCH = T // 512            # 4 i-chunks of 512
SCALE = 1.0 / 8.0         # 1/sqrt(D)


def build_body(ctx, tc, xT, wq, wk, wv, wp, tri, ident, onesd, out):
    nc = tc.nc

    consts = ctx.enter_context(tc.tile_pool(name="consts", bufs=1))
    acts = ctx.enter_context(tc.tile_pool(name="acts", bufs=1))
    pp = ctx.enter_context(tc.tile_pool(name="pp", bufs=4))
    rcp = ctx.enter_context(tc.tile_pool(name="rcp", bufs=2))
    y2p = ctx.enter_context(tc.tile_pool(name="y2p", bufs=8))
    obp = ctx.enter_context(tc.tile_pool(name="obp", bufs=5))
    ps_mm = ctx.enter_context(tc.tile_pool(name="ps_mm", bufs=2, space="PSUM"))
    ps_pj = ctx.enter_context(tc.tile_pool(name="ps_pj", bufs=2, space="PSUM"))
    ps_ot = ctx.enter_context(tc.tile_pool(name="ps_ot", bufs=2, space="PSUM"))

    # ---- SBUF tiles ----
    wq_sb = consts.tile([128, KT * OL], BF16)
    wk_sb = consts.tile([128, KT * OL], BF16)
    wv_sb = consts.tile([128, KT * OL], BF16)
    xT_sb = consts.tile([128, KT * T], BF16)
    tri_sb = consts.tile([128, 128], BF16)
    ident_sb = consts.tile([128, 128], BF16)
    wp_sb = consts.tile([128, 2 * C], BF16)

    q_sb = acts.tile([128, 2 * T], BF16)
    k_sb = acts.tile([128, 2 * T], BF16)
    v_sb = acts.tile([128, TT * HL * (D + 1)], BF16)
    yt_sb = acts.tile([128, 2 * T], BF16)

    # ---- input DMA ----
    # Host supplies all tensors pre-tiled to the exact SBUF layouts, so every
    # load is a contiguous full-rate DMA.  x is chunk-major: [128, NCH, KT, 512].
    def load_w(w_sb, w_d, k0, k1):
        nc.sync.dma_start(w_sb[:, k0 * OL:k1 * OL], w_d[:, k0 * OL:k1 * OL])

    def load_x(n, k0, k1):
        nc.sync.dma_start(
            xT_sb[:, (n * KT + k0) * 512:(n * KT + k1) * 512],
            xT[:, (n * KT + k0) * 512:(n * KT + k1) * 512])

    load_w(wq_sb, wq, 0, 2)
    load_x(0, 0, 2)
    load_w(wq_sb, wq, 2, KT)
    load_x(0, 2, 5)
    load_x(0, 5, KT)
    load_w(wv_sb, wv, 0, KT)
    load_w(wk_sb, wk, 0, KT)
    load_x(1, 0, KT)
    nc.sync.dma_start(tri_sb[:], tri[:, :])
    nc.sync.dma_start(ident_sb[:], ident[:, :])
    # ones column (index D) of every [t-tile, head] V block
    v_ones = v_sb[:].rearrange("p (g c) -> p g c", c=D + 1)[:, :, D]
    nc.sync.dma_start(v_ones, onesd[:, :])
    load_x(2, 0, KT)
    load_x(3, 0, KT)
    nc.sync.dma_start(wp_sb[:], wp[:, :])

    # ---- qkv / proj emission units ----
    def qk_unit(n, which, m):
        w_sb, dst = (wq_sb, q_sb) if which == 0 else (wk_sb, k_sb)
        ps = ps_pj.tile([128, 512], F32, tag="pj")
        for k in range(KT):
            nc.tensor.matmul(
                ps[:],
                w_sb[:, k * OL + m * 128: k * OL + (m + 1) * 128],
                xT_sb[:, (n * KT + k) * 512:(n * KT + k + 1) * 512],
                start=(k == 0), stop=(k == KT - 1),
            )
        nc.vector.tensor_copy(dst[:, m * T + n * 512: m * T + (n + 1) * 512], ps[:])

    def v_unit(t):
        ps = ps_pj.tile([128, OL], F32, tag="pj")
        n, tl = divmod(t, 4)
        for k in range(KT):
            nc.tensor.matmul(
                ps[:],
                xT_sb[:, (n * KT + k) * 512 + tl * 128:
                       (n * KT + k) * 512 + (tl + 1) * 128],
                wv_sb[:, k * OL:(k + 1) * OL],
                start=(k == 0), stop=(k == KT - 1),
            )
        dst = v_sb[:, t * HL * (D + 1): (t + 1) * HL * (D + 1)]
        dst = dst.rearrange("p (h c) -> p h c", h=HL)[:, :, 0:D]
        nc.vector.tensor_copy(dst, ps[:].rearrange("p (h c) -> p h c", h=HL))

    _ob_state = {}

    def proj_unit(t, n2):
        ps = ps_pj.tile([128, 512], F32, tag="pj")
        for kk in range(2):
            nc.tensor.matmul(
                ps[:],
                yt_sb[:, kk * T + t * 128: kk * T + (t + 1) * 128],
                wp_sb[:, kk * C + n2 * 512: kk * C + (n2 + 1) * 512],
                start=(kk == 0), stop=(kk == 1),
            )
        # pair two t-tiles per output store to halve DMA dispatch count,
        # except the last two tiles (tail latency: let t14's store overlap
        # t15's projection)
        if t >= TT - 2:
            if n2 == 0:
                ob = obp.tile([128, 1024], BF16, tag="ob", name=f"ob_s{t}")
                nc.scalar.copy(ob[:, 0:512], ps[:])
                _ob_state[t] = ob
            else:
                ob = _ob_state.pop(t)
                nc.vector.tensor_copy(ob[:, 512:1024], ps[:])
                nc.scalar.dma_start(out[t * 128:(t + 1) * 128, :], ob[:])  # tail singles (t14/t15)
            return
        tp = t // 2
        ob = _ob_state.get(tp)
        if ob is None:
            ob = obp.tile([128, 2, 1024], BF16, tag="ob", name=f"ob_{tp}")
            _ob_state[tp] = ob
        cp = nc.scalar.copy if t >= 12 and n2 == 0 else nc.vector.tensor_copy
        if n2 == 0:
            cp(ob[:, t % 2, 0:512], ps[:])
        else:
            cp(ob[:, t % 2, 512:1024], ps[:])
            if t % 2 == 1:
                dma_q = nc.scalar if t >= 13 else nc.sync
                dma_q.dma_start(
                    out[(t - 1) * 128:(t + 1) * 128, :]
                    .rearrange("(g p) c -> p g c", g=2),
                    ob[:])
                del _ob_state[tp]

    def qk_units(n):
        return [(lambda n=n, w=w, m=m: qk_unit(n, w, m))
                for w in range(2) for m in range(2)]

    def v_units(n):
        return [(lambda t=t: v_unit(t)) for t in range(4 * n, 4 * n + 4)]

    # ---- attention ----
    # state per (h, ic): ps_o [128 i, 4 itl, 65] accumulators + rc [128, 4]
    # state per ic: y4 [128 i, 4 itl x 256 d2] normalized outputs (all heads)
    _y4 = {}

    def attn_head_blocks(ic, h, post_diag0=lambda: None):
        pair, hl = h // 2, h % 2
        pb = 64 * hl
        mo = pair * T
        state = {}

        def ot_mms(kt, pieces, p_t):
            # pieces: list of (itl, col_off); emits stationary-P matmuls.
            # PSUM start_tensor_calc zeroes the whole 2KB bank (lazily), so
            # only the FIRST matmul into the bank may set start=True; the
            # other i-tile groups' first writes land on pending-zero bytes
            # and are zeroed by the hardware as part of that one start.
            for idx, (itl, co) in enumerate(pieces):
                nc.tensor.matmul(
                    state["ps_o"][:, itl, 0:D + 1],
                    p_t[:, co:co + 128],
                    v_sb[:, (kt * HL + h) * (D + 1):(kt * HL + h + 1) * (D + 1)],
                    start=(kt == 0 and idx == 0), stop=(kt == 4 * ic + itl),
                    skip_group_check=True,
                )

        def norm2(a):
            # normalize i-tiles a, a+1 in one reciprocal + one broadcast mul
            if a == 0:
                state["rc"] = rcp.tile([128, 4], F32, tag="rc", name=f"rc_{ic}_{h}")
            rc = state["rc"]
            ps_o = state["ps_o"]
            y4 = _y4.get(ic)
            if y4 is None:
                y4 = y2p.tile([128, 1024], BF16, tag="y4", name=f"y4_{ic}")
                _y4[ic] = y4
            nc.vector.reciprocal(rc[:, a:a + 2], ps_o[:, a:a + 2, D])
            dst = (y4[:].rearrange("p (i c) -> p i c", i=4)
                   [:, a:a + 2, h * 64:(h + 1) * 64])
            nc.vector.tensor_mul(dst, ps_o[:, a:a + 2, 0:D],
                                 rc[:, a:a + 2].to_broadcast([128, 2, D]))

        def transpose(itl):
            # one XBAR transpose covers both head pairs: [128 i, 256 d] ->
            # [256 d, 128 i] landing as kk=0/1 blocks of yt.  For the LAST
            # chunk the XBAR dispatch latency (~1.5us) would sit on the
            # critical tail, so transpose on the PE instead (identity-matmul,
            # 53ns) with ACT+DVE copies out of PSUM.
            t = 4 * ic + itl
            y4 = _y4[ic]
            if ic == 3:
                ps_t = ps_pj.tile([128, 2, 128], BF16, tag="pj", name=f"ps_t_{itl}")
                for j in range(2):
                    nc.tensor.matmul(
                        ps_t[:, j, :],
                        y4[:, itl * 256 + j * 128: itl * 256 + (j + 1) * 128],
                        ident_sb[:],
                        is_transpose=True,
                        start=(j == 0), stop=(j == 1),
                        skip_group_check=True,
                    )
                nc.scalar.copy(yt_sb[:, 0 * T + t * 128: 0 * T + (t + 1) * 128],
                               ps_t[:, 0, :])
                nc.vector.tensor_copy(yt_sb[:, 1 * T + t * 128: 1 * T + (t + 1) * 128],
                                      ps_t[:, 1, :])
            else:
                dst = yt_sb[:].rearrange("p (g t) -> p g t", g=2)[:, :, t * 128:(t + 1) * 128]
                nc.sync.dma_start_transpose(dst, y4[:, itl * 256:(itl + 1) * 256])
            if itl == 3:
                del _y4[ic]


        def fp_scores(tja):
            if tja == 0:
                state["ps_o"] = ps_ot.tile([128, 4, 128], F32, tag="ot",
                                           name=f"ps_o_{ic}_{h}")
            qh = q_sb[pb:pb + 64, mo:mo + T]
            kh = k_sb[pb:pb + 64, mo:mo + T]
            ps_s = ps_mm.tile([128, 1024], F32, tag="mm", name=f"ps_s_{ic}_{h}")
            for j in range(2):
                nc.tensor.matmul(
                    ps_s[:, j * 512:(j + 1) * 512],
                    kh[:, (tja + j) * 128:(tja + j + 1) * 128],
                    qh[:, ic * 512:(ic + 1) * 512],
                    start=True, stop=True,
                    skip_group_check=True,
                )
            p_t = pp.tile([128, 1024], BF16, tag="p", name=f"p_t_{ic}_{h}")
            nc.scalar.activation(p_t[:], ps_s[:], mybir.ActivationFunctionType.Exp,
                                 scale=SCALE)
            state.setdefault("pq", []).append((tja, p_t))

        def fp_ot():
            tja, p_t = state["pq"].pop(0)
            for j in range(2):
                ot_mms(tja + j, [(itl, j * 512 + itl * 128) for itl in range(4)],
                       p_t)

        def diag0a():
            if ic == 0:
                state["ps_o"] = ps_ot.tile([128, 4, 128], F32, tag="ot",
                                           name=f"ps_o_{ic}_{h}")
            qh = q_sb[pb:pb + 64, mo:mo + T]
            kh = k_sb[pb:pb + 64, mo:mo + T]
            kt = 4 * ic
            ps_s = ps_mm.tile([128, 1024], F32, tag="mm", name=f"ps_d0_{ic}_{h}")
            nc.tensor.matmul(
                ps_s[:, 0:512], kh[:, kt * 128:(kt + 1) * 128],
                qh[:, ic * 512:(ic + 1) * 512],
                start=True, stop=True, skip_group_check=True)
            nc.tensor.matmul(
                ps_s[:, 512:896], kh[:, (kt + 1) * 128:(kt + 2) * 128],
                qh[:, ic * 512 + 128:(ic + 1) * 512],
                start=True, stop=True, skip_group_check=True)
            p_t = pp.tile([128, 1024], BF16, tag="p", name=f"p_d0_{ic}_{h}")
            nc.scalar.activation(p_t[:, 0:896], ps_s[:, 0:896],
                                 mybir.ActivationFunctionType.Exp, scale=SCALE)
            nc.gpsimd.tensor_mul(p_t[:, 0:128], p_t[:, 0:128], tri_sb[:])
            nc.gpsimd.tensor_mul(p_t[:, 512:640], p_t[:, 512:640], tri_sb[:])
            state["p_d0"] = p_t

        def diag0b():
            kt = 4 * ic
            p_t = state.pop("p_d0")
            ot_mms(kt, [(itl, itl * 128) for itl in range(4)], p_t)
            ot_mms(kt + 1, [(itl, 512 + (itl - 1) * 128) for itl in (1, 2, 3)], p_t)
            norm2(0)
            if h == 3:
                transpose(0)
                transpose(1)
                post_diag0()

        def diag2a():
            qh = q_sb[pb:pb + 64, mo:mo + T]
            kh = k_sb[pb:pb + 64, mo:mo + T]
            kt = 4 * ic + 2
            ps_s = ps_mm.tile([128, 1024], F32, tag="mm", name=f"ps_d2_{ic}_{h}")
            nc.tensor.matmul(
                ps_s[:, 0:256], kh[:, kt * 128:(kt + 1) * 128],
                qh[:, ic * 512 + 256:(ic + 1) * 512],
                start=True, stop=True, skip_group_check=True)
            nc.tensor.matmul(
                ps_s[:, 256:384], kh[:, (kt + 1) * 128:(kt + 2) * 128],
                qh[:, ic * 512 + 384:(ic + 1) * 512],
                start=True, stop=True, skip_group_check=True)
            p_t = pp.tile([128, 1024], BF16, tag="p", name=f"p_d2_{ic}_{h}")
            nc.scalar.activation(p_t[:, 0:384], ps_s[:, 0:384],
                                 mybir.ActivationFunctionType.Exp, scale=SCALE)
            nc.gpsimd.tensor_mul(p_t[:, 0:128], p_t[:, 0:128], tri_sb[:])
            nc.gpsimd.tensor_mul(p_t[:, 256:384], p_t[:, 256:384], tri_sb[:])
            state["p_d2"] = p_t

        def diag2b():
            kt = 4 * ic + 2
            p_t = state.pop("p_d2")
            ot_mms(kt, [(2, 0), (3, 128)], p_t)
            ot_mms(kt + 1, [(3, 256)], p_t)
            norm2(2)
            if h == 3:
                transpose(2)
                transpose(3)

        # software-pipelined emission: scores of pair n+1 land on the PE
        # queue before the OT of pair n, so the PE never sits on an exp wait
        blocks = []
        npair = 2 * ic
        for p in range(npair):
            blocks.append((lambda p=p: fp_scores(2 * p), 427, 1040))
            if p >= 1:
                blocks.append((fp_ot, 216, 0))
        blocks.append((diag0a, 373, 900))
        if npair >= 1:
            blocks.append((fp_ot, 216, 0))
        blocks += [(diag2a, 160, 460), (diag0b, 190, 0), (diag2b, 80, 0)]
        return blocks

    def emit_interleaved(blocks, fillers):
        # Deficit-driven: blocks are (callable, pe_ns, act_ns); emit a filler
        # whenever the ACT engine is cumulatively ahead of the PE work queued,
        # so the PE never starves while exps drain.
        fi = 0
        pe_t, act_t = 0.0, 0.0
        for blk, pe_ns, act_ns in blocks:
            blk()
            pe_t += pe_ns
            act_t += act_ns
            while fi < len(fillers) and act_t > pe_t:
                f, f_pe = fillers[fi]
                f()
                pe_t += f_pe
                fi += 1
        while fi < len(fillers):
            fillers[fi][0]()
            fi += 1

    def proj_units(ic, t0=0, t1=4):
        return [(lambda t=t, n2=n2: proj_unit(t, n2))
                for t in range(4 * ic + t0, 4 * ic + t1) for n2 in range(2)]

    # schedule: qkv(0) first; chunk ic interleaves qkv(ic+1) and deferred
    # projection work as PE filler.  The late chunks are locally ACT-bound
    # (exp is the binding resource there), so they get extra PE work: the
    # chunk-3 v-units, the back half of proj(1), and proj(2).  The chunk-3
    # projections for t12/t13 slide into head 3's diag0 slot, leaving only
    # t14/t15 (plus stores) as the serial tail.
    for u in qk_units(0) + v_units(0):
        u()
    for ic in range(NCH):
        blocks = []
        if ic == 3:
            blocks += [(u, 853, 0) for u in v_units(3)]
        post = lambda: None
        if ic == 3:
            def post():
                for u in proj_units(3, 0, 2):
                    u()
        for h in range(4):
            blocks += attn_head_blocks(ic, h, post_diag0=(post if h == 3 else (lambda: None)))
        fill = []
        if ic < 2:
            fill += [(u, 1707) for u in qk_units(ic + 1)]
            fill += [(u, 853) for u in v_units(ic + 1)]
        elif ic == 2:
            fill += [(u, 1707) for u in qk_units(3)]
            fill += [(u, 427) for u in proj_units(0)]
        elif ic == 3:
            fill += [(u, 427) for u in proj_units(1) + proj_units(2)]
        emit_interleaved(blocks, fill)
    for u in proj_units(3, 2, 4):
        u()


def build_program(reps=1):
    from contextlib import ExitStack

    nc = bacc.Bacc("TRN2", target_bir_lowering=False, debug=False)
    # all inputs pre-tiled by the host to SBUF layouts (partition dim first)
    xT = nc.dram_tensor("xT", [128, NCH * KT * 512], BF16, kind="ExternalInput").ap()
    wq = nc.dram_tensor("wq", [128, KT * OL], BF16, kind="ExternalInput").ap()
    wk = nc.dram_tensor("wk", [128, KT * OL], BF16, kind="ExternalInput").ap()
    wv = nc.dram_tensor("wv", [128, KT * OL], BF16, kind="ExternalInput").ap()
    wp = nc.dram_tensor("wp", [128, 2 * C], BF16, kind="ExternalInput").ap()
    tri = nc.dram_tensor("tri", [128, 128], BF16, kind="ExternalInput").ap()
    ident = nc.dram_tensor("ident", [128, 128], BF16, kind="ExternalInput").ap()
    onesd = nc.dram_tensor("onesd", [128, TT * HL], BF16, kind="ExternalInput").ap()
    out = nc.dram_tensor("out", [T, C], BF16, kind="ExternalOutput").ap()

    with tile.TileContext(nc) as tc:
        for _ in range(reps):
            with ExitStack() as ctx:
                        build_body(ctx, tc, xT, wq, wk, wv, wp, tri, ident, onesd, out)
    nc.compile()
    return nc


def quant_weight_np(w):
    scale = max(np.mean(np.abs(w), dtype=np.float32), np.float32(1e-8))
    return (np.clip(np.round(w / scale), -2.0, 2.0) * scale).astype(np.float32)


def _tile_w(w):
    # [C, OL] -> [128, KT*OL]: SBUF layout, k-tile major along free dim
    return np.ascontiguousarray(
        w.reshape(KT, 128, OL).transpose(1, 0, 2).reshape(128, KT * OL))


def _tile_x(xTb):
    # [C, T] -> [128, NCH*KT*512]: chunk-major, then k-tile, then 512 cols
    v = xTb.reshape(KT, 128, NCH, 512)        # [k, p, n, col]
    return np.ascontiguousarray(
        v.transpose(1, 2, 0, 3).reshape(128, NCH * KT * 512))


def make_in_maps(x, w_attn, w_proj):
    wq_f = quant_weight_np(w_attn)
    wp_f = quant_weight_np(w_proj)
    tri = np.triu(np.ones((128, 128), dtype=np.float32))
    in_maps = []
    for core in range(8):
        b, g = divmod(core, 4)
        sl = slice(g * OL, (g + 1) * OL)
        wp_l = wp_f[:, sl].T                 # [OL, C]
        wp_t = np.ascontiguousarray(
            wp_l.reshape(2, 128, C).transpose(1, 0, 2).reshape(128, 2 * C))
        in_maps.append({
            "xT": _tile_x(x[b].T).astype(ml_dtypes.bfloat16),
            "wq": _tile_w(wq_f[0 * C:1 * C][sl].T).astype(ml_dtypes.bfloat16),
            "wk": _tile_w(wq_f[1 * C:2 * C][sl].T).astype(ml_dtypes.bfloat16),
            "wv": _tile_w(wq_f[2 * C:3 * C][sl].T).astype(ml_dtypes.bfloat16),
            "wp": wp_t.astype(ml_dtypes.bfloat16),
            "tri": tri.astype(ml_dtypes.bfloat16),
            "ident": np.eye(128, dtype=np.float32).astype(ml_dtypes.bfloat16),
            "onesd": np.ones((128, TT * HL), dtype=ml_dtypes.bfloat16),
        })
    return in_maps


_CACHED_NC = None


def kernel(x, w_attn, w_proj):
    global _CACHED_NC
    if _CACHED_NC is None:
        _CACHED_NC = build_program()
    in_maps = make_in_maps(np.asarray(x, dtype=np.float32),
                           np.asarray(w_attn, dtype=np.float32),
                           np.asarray(w_proj, dtype=np.float32))
    res = run_bass_kernel_spmd(_CACHED_NC, in_maps, list(range(8)))
    out = np.zeros((B, T, C), dtype=np.float32)
    for core in range(8):
        b = core // 4
        out[b] += res.results[core]["out"].astype(np.float32)
    return out
